# revision 22
# baseline (speedup 1.0000x reference)
# Dense-MoE (all experts active) Trainium2 kernel, expert-parallel over 8
# NeuronCores. Each core computes its expert's 2-layer MLP over all tokens:
#   fe_e = gelu(h @ W1[e] + b1[e]) @ (probs[e] * W2[e]) + probs[e] * b2[e]
# then chunked ReduceScatter(add) across the 8 cores sums the expert
# contributions; the host reassembles the full [B, D] output.
#
# Default path (build_v2): fp16 matmuls, activations transposed on-chip
# (hT [IN, tokens] pre-transposed on host). Two token groups of 2048:
#   L1: for each H-chunk m, one stationary W1 block serves a 4-wide
#       token-tile PSUM sweep (amortizes ldweights 4x); gelu+b1 fused in
#       the ACT read.
#   L2: operands swapped vs the obvious layout -- stationary = W2 block,
#       psum partition = output-D chunk -- so b2 rides the ACT bias port
#       and the per-dd output is produced transposed (feT), again with a
#       4-wide token sweep per stationary.
#   Output: bf16 ReduceScatter per dd-pair (8 ops x 1 MB). fp16
#   collectives hit a ~650 us slow path on this runtime; bf16 runs at
#   full rate. f32 via BEST_CFG["fe_dt"]=F32 if more margin is wanted.
# Measured (chained-dispatch slope minus 224 us fixed overhead):
#   v1 (original): ~609 us;  v2: ~475-505 us;  rel err 5.4e-3 (<2e-2).
# Legacy v1 path kept under MOE_MM_DTYPE=float16/float32r.
import os
import sys

sys.path.insert(0, "/opt/trn_rl_repo")

import numpy as np

import concourse.mybir as mybir
from concourse import bacc, tile
from concourse import masks

B, E, IN, H, D = 4096, 8, 1024, 2048, 1024
NCORES = 8
P = 128
BT = 512                  # tokens per B-tile
NBT = B // BT             # 8 B-tiles
NSUB = BT // P            # 4 token sub-tiles per B-tile
KC1 = IN // P             # 8 contraction chunks, layer 1
MC1 = H // P              # 16 H chunks
ND = D // 512             # 2 output column slices of 512
HALF = 2 * P              # 256 rows per ReduceScatter chunk (1 MB)
RS_ROWS = HALF // NCORES  # 32 rows each core receives per RS chunk
NCHUNK = NBT * 2          # 16 RS chunks

F32 = mybir.dt.float32

_CACHE = {}


def build(mm_dtype_name="float16", nbt=NBT, use_collective=True):
    mm_dt = getattr(mybir.dt, mm_dtype_name)
    bf16 = mybir.dt.size(mm_dt) == 2  # 2-byte path: bf16 or fp16
    nc = bacc.Bacc("TRN2", target_bir_lowering=False)

    if bf16:
        hT = nc.declare_dram_parameter("ht", [IN, nbt * BT], mm_dt, isOutput=False)
    else:
        h = nc.declare_dram_parameter("h", [nbt * BT, IN], F32, isOutput=False)
    w1 = nc.declare_dram_parameter("w1", [IN, H], mm_dt, isOutput=False)
    b1t = nc.declare_dram_parameter("b1t", [P, MC1], F32, isOutput=False)
    w2 = nc.declare_dram_parameter("w2", [H, D], mm_dt, isOutput=False)
    b2b = nc.declare_dram_parameter("b2b", [P, D], F32, isOutput=False)
    out_rows = nbt * BT // NCORES if use_collective else nbt * BT
    out = nc.declare_dram_parameter("out", [out_rows, D], F32, isOutput=True)

    with tile.TileContext(nc) as tc:
        with (
            tc.tile_pool(name="weights", bufs=1) as wpool,
            tc.tile_pool(name="consts", bufs=1) as cpool,
            tc.tile_pool(name="hraw", bufs=2) as hraw_pool,
            tc.tile_pool(name="ht", bufs=(3 if mybir.dt.size(mm_dt) == 2 else 2)) as ht_pool,
            tc.tile_pool(name="hid", bufs=(2 if mybir.dt.size(mm_dt) == 2 else 1)) as hid_pool,
            tc.tile_pool(name="fe", bufs=(2 if mybir.dt.size(mm_dt) == 2 else 1)) as fe_pool,
            tc.tile_pool(name="tp_ps", bufs=(1 if bf16 else 2),
                         space="PSUM") as tp_psum,
            tc.tile_pool(name="l1_ps", bufs=(3 if bf16 else 2),
                         space="PSUM") as l1_psum,
            tc.tile_pool(name="l2_ps", bufs=4, space="PSUM") as l2_psum,
            tc.tile_pool(name="dram", bufs=4, space="DRAM") as dram_pool,
        ):
            hr_pre = []
            ht0 = None
            if bf16:
                ht0 = ht_pool.tile([P, KC1 * BT], mm_dt, tag="ht")
            if not bf16:
                ident = cpool.tile([P, P], F32, tag="ident")
                masks.make_identity(nc, ident[:])

                # prefetch the first two h sub-tiles ahead of the weight slabs
                # so the transpose pipeline starts immediately
                def prefetch_hr(s):
                    hr = hraw_pool.tile([P, IN], F32, tag="hr")
                    nc.sync.dma_start(hr[:], h[s * P:(s + 1) * P, :])
                    hr_pre.append(hr)

                prefetch_hr(0)
                prefetch_hr(1)

            # per-slab weight tiles so the first matmuls depend only on their
            # own slab's DMA; first-tile h loads interleaved so the transpose
            # pipeline is never starved behind 16MB of weights
            w1_sb = []
            for k in range(KC1):
                if bf16:
                    # pair each W1 slab with the matching first-tile hT chunk
                    # so L1's k-accumulation can start as soon as pair 0 lands
                    nc.sync.dma_start(
                        ht0[:, k * BT:(k + 1) * BT],
                        hT[k * P:(k + 1) * P, 0:BT],
                    )
                t_ = wpool.tile([P, H], mm_dt, tag=f"w1_{k}")
                nc.sync.dma_start(t_[:], w1[k * P:(k + 1) * P, :])
                w1_sb.append(t_)
                if k == 3 and not bf16:
                    prefetch_hr(2)
            if not bf16:
                prefetch_hr(3)
            b1_sb = cpool.tile([P, MC1], F32, tag="b1")
            nc.sync.dma_start(b1_sb[:], b1t[:])
            w2_sb = []
            for m in range(MC1):
                t_ = wpool.tile([P, D], mm_dt, tag=f"w2_{m}")
                nc.sync.dma_start(t_[:], w2[m * P:(m + 1) * P, :])
                w2_sb.append(t_)
            b2_sb = cpool.tile([P, D], F32, tag="b2")
            nc.sync.dma_start(b2_sb[:], b2b[:])

            for t in range(nbt):
                # --- transpose this tile's h rows into hT ---
                # ht chunk k (IN rows k*128..) lives at columns [k*BT, (k+1)*BT)
                if bf16 and t == 0:
                    ht = ht0
                else:
                    ht = ht_pool.tile([P, KC1 * BT], mm_dt, tag="ht")
                if bf16 and t > 0:
                    # hT comes pre-transposed from the host: plain strided DMA
                    for k in range(KC1):
                        nc.sync.dma_start(
                            ht[:, k * BT:(k + 1) * BT],
                            hT[k * P:(k + 1) * P, t * BT:(t + 1) * BT],
                        )
                elif not bf16:
                    for s in range(NSUB):
                        if t == 0:
                            hr = hr_pre[s]
                        else:
                            hr = hraw_pool.tile([P, IN], F32, tag="hr")
                            nc.sync.dma_start(
                                hr[:], h[t * BT + s * P: t * BT + (s + 1) * P, :]
                            )
                        for k in range(KC1):
                            tp = tp_psum.tile([P, P], F32, tag="tp")
                            nc.tensor.transpose(
                                tp[:], hr[:, k * P:(k + 1) * P], ident[:]
                            )
                            nc.vector.tensor_copy(
                                ht[:, k * BT + s * P: k * BT + (s + 1) * P], tp[:]
                            )

                # --- layer 1: hidT chunk m = (W1 block).T @ hT, + b1, gelu ---
                # hid is split in two halves so layer 2's sweep releases the
                # first half early for the next tile's evictions
                hidA = hid_pool.tile([P, (MC1 // 2) * BT], mm_dt, tag="hidA")
                hidB = hid_pool.tile([P, (MC1 // 2) * BT], mm_dt, tag="hidB")

                def hid_slice(m, lo, hi):
                    half_t = hidA if m < MC1 // 2 else hidB
                    mm_ = m % (MC1 // 2)
                    return half_t[:, mm_ * BT + lo: mm_ * BT + hi]

                for m in range(MC1):
                    ps = l1_psum.tile([P, BT], F32, tag="l1")
                    for k in range(KC1):
                        nc.tensor.matmul(
                            ps[:],
                            w1_sb[k][:, m * P:(m + 1) * P],
                            ht[:, k * BT:(k + 1) * BT],
                            start=(k == 0),
                            stop=(k == KC1 - 1),
                        )
                    nc.scalar.activation(
                        hid_slice(m, 0, BT),
                        ps[:],
                        mybir.ActivationFunctionType.Gelu,
                        bias=b1_sb[:, m:m + 1],
                        scale=1.0,
                    )

                # --- layer 2 + chunked ReduceScatter (2MB per tile,
                # tapering to 2x1MB on the final tile for a short tail) ---
                nhalves = 2 if t == nbt - 1 else 1
                subs_per_chunk = NSUB // nhalves
                for half in range(nhalves):
                    fe_chunk = dram_pool.tile(
                        [subs_per_chunk * P, D], F32, tag="fe_dram"
                    )
                    for si in range(subs_per_chunk):
                        s = half * subs_per_chunk + si
                        # both d-slices accumulate together: the second matmul
                        # of each pair reuses the stationary hid block already
                        # in the PE array (ldweights=False) instead of
                        # reloading it
                        ps_a = l2_psum.tile([P, 512], F32, tag="l2")
                        ps_b = l2_psum.tile([P, 512], F32, tag="l2")
                        pss = [ps_a, ps_b]
                        for m in range(MC1):
                            hs = hid_slice(m, s * P, (s + 1) * P)
                            for d in range(ND):
                                mi = nc.tensor.matmul(
                                    pss[d][:],
                                    hs,
                                    w2_sb[m][:, d * 512:(d + 1) * 512],
                                    start=(m == 0),
                                    stop=(m == MC1 - 1),
                                )
                                if d > 0:
                                    mi.ins.ldweights = False
                        for d in range(ND):
                            fe_sb = fe_pool.tile([P, 512], F32, tag="fe_sb")
                            nc.vector.tensor_add(
                                fe_sb[:], pss[d][:],
                                b2_sb[:, d * 512:(d + 1) * 512]
                            )
                            nc.sync.dma_start(
                                fe_chunk[si * P:(si + 1) * P,
                                         d * 512:(d + 1) * 512],
                                fe_sb[:],
                            )

                    chunk_rows = subs_per_chunk * P // NCORES
                    row0 = (t * BT + half * subs_per_chunk * P) // NCORES
                    if use_collective:
                        rs_chunk = dram_pool.tile(
                            [chunk_rows, D], F32, tag="rs_dram"
                        )
                        nc.gpsimd.collective_compute(
                            "ReduceScatter",
                            mybir.AluOpType.add,
                            replica_groups=[list(range(NCORES))],
                            ins=[fe_chunk[:]],
                            outs=[rs_chunk[:]],
                        )
                        nc.sync.dma_start(
                            out[row0:row0 + chunk_rows, :], rs_chunk[:]
                        )
                    else:
                        r0 = t * BT + half * subs_per_chunk * P
                        nc.sync.dma_start(
                            out[r0:r0 + subs_per_chunk * P, :], fe_chunk[:]
                        )

    nc.finalize()
    return nc


TG = 2048                 # tokens per group (v2)
NG = B // TG              # 2 groups
NTS = TG // BT            # 4 token sub-tiles of 512 per group
NDD = D // P              # 8 output d-chunks of 128
F16 = mybir.dt.float16


def _rs_chunks(ndd_rs):
    """Per-group RS chunk schedule: list (per g) of (dd0, ndds, out_row0).
    The final chunk of the final group is split into single-dd chunks to
    shorten the drain tail."""
    sched = []
    row = 0
    for g in range(NG):
        chunks = []
        dd = 0
        while dd < NDD:
            ndds = ndd_rs
            if g == NG - 1 and dd >= NDD - ndd_rs and ndd_rs > 1:
                ndds = 1
            chunks.append((dd, ndds, row))
            row += ndds * P // NCORES
            dd += ndds
        sched.append(chunks)
    return sched


def build_v2(fe_dt=F16, use_rs=True, ndd_rs=2, shared_psum=False):
    """fp16 kernel, v2: stationary-reuse token sweeps in both layers,
    L2 emits transposed output (psum partition = d) so the b2 bias rides
    the ACT instruction, ReduceScatter per dd-pair."""
    nc = bacc.Bacc("TRN2", target_bir_lowering=False)

    hT = nc.declare_dram_parameter("ht", [IN, B], F16, isOutput=False)
    w1 = nc.declare_dram_parameter("w1", [IN, H], F16, isOutput=False)
    b1t = nc.declare_dram_parameter("b1t", [P, MC1], F32, isOutput=False)
    w2 = nc.declare_dram_parameter("w2", [H, D], F16, isOutput=False)
    b2t = nc.declare_dram_parameter("b2t", [P, NDD], F32, isOutput=False)
    # out rows: (g, dd) chunk -> 16 d-rows per core, 2048 tokens
    out_rows = NG * NDD * (P // NCORES) if use_rs else NG * NDD * P
    out = nc.declare_dram_parameter("out", [out_rows, TG], fe_dt,
                                    isOutput=True)

    with tile.TileContext(nc) as tc:
        with (
            tc.tile_pool(name="weights", bufs=1) as wpool,
            tc.tile_pool(name="consts", bufs=1) as cpool,
            tc.tile_pool(name="ht", bufs=2) as ht_pool,
            tc.tile_pool(name="hid", bufs=1) as hid_pool,
            tc.tile_pool(name="fe", bufs=4) as fe_pool,
            tc.tile_pool(name="l1_ps", bufs=(8 if shared_psum else 4),
                         space="PSUM") as l1_psum,
            tc.tile_pool(name="dram", bufs=6, space="DRAM") as dram_pool,
        ):
            # --- weight / bias / first-group hT loads (interleaved) ---
            # Startup staging: the first m-chunks of L1 run as 2-wide
            # half-sweeps over tokens 0:TG/2, so the critical path to the
            # first matmuls is one hT token-half (2 MB) + one w1 column
            # quarter (1 MB) instead of the full 8 MB.
            ht0 = ht_pool.tile([P, KC1, TG], F16, tag="ht", name="ht_g0")
            w1_sb = []
            for k in range(KC1):
                nc.sync.dma_start(
                    ht0[:, k, 0:TG // 2], hT[k * P:(k + 1) * P, 0:TG // 2])
                t_ = wpool.tile([P, H], F16, tag=f"w1_{k}", name=f"w1_{k}")
                nc.sync.dma_start(
                    t_[:, 0:H // 4], w1[k * P:(k + 1) * P, 0:H // 4])
                w1_sb.append(t_)
            b1_sb = cpool.tile([P, MC1], F32, tag="b1", name="b1_sb")
            nc.sync.dma_start(b1_sb[:], b1t[:])
            for k in range(KC1):
                nc.sync.dma_start(
                    ht0[:, k, TG // 2:TG],
                    hT[k * P:(k + 1) * P, TG // 2:TG])
                nc.sync.dma_start(
                    w1_sb[k][:, H // 4:H // 2],
                    w1[k * P:(k + 1) * P, H // 4:H // 2])
            for k in range(KC1):
                nc.sync.dma_start(
                    w1_sb[k][:, H // 2:H],
                    w1[k * P:(k + 1) * P, H // 2:H])
            w2_sb = []
            for m in range(MC1):
                t_ = wpool.tile([P, D], F16, tag=f"w2_{m}", name=f"w2_{m}")
                nc.sync.dma_start(t_[:], w2[m * P:(m + 1) * P, :])
                w2_sb.append(t_)
            b2_sb = cpool.tile([P, NDD], F32, tag="b2", name="b2_sb")
            nc.sync.dma_start(b2_sb[:], b2t[:])

            ht_tiles = [ht0]
            for g in range(1, NG):
                htg = ht_pool.tile([P, KC1, TG], F16, tag="ht", name=f"ht_g{g}")
                for k in range(KC1):
                    nc.sync.dma_start(
                        htg[:, k], hT[k * P:(k + 1) * P, g * TG:(g + 1) * TG])
                ht_tiles.append(htg)

            for g in range(NG):
                htg = ht_tiles[g]
                hid = hid_pool.tile([P, MC1, TG], F16, tag="hid",
                                    name=f"hid_g{g}")

                # --- L1: hid[m, :] = gelu(W1[:,m-chunk].T @ ht + b1) ---
                for m in range(MC1):
                    # group 0's first two m-chunks run as 2-wide
                    # half-sweeps so they only depend on the first hT
                    # token-half (see startup staging above)
                    halves = 2 if (g == 0 and m < 2) else 1
                    width = NTS // halves
                    for hf in range(halves):
                        pss = [
                            l1_psum.tile([P, BT], F32, tag="l1",
                                         name=f"l1_{g}_{m}_{hf}_{t}")
                            for t in range(width)
                        ]
                        t0 = hf * width
                        for k in range(KC1):
                            stat = w1_sb[k][:, m * P:(m + 1) * P]
                            for t in range(width):
                                mi = nc.tensor.matmul(
                                    pss[t][:], stat,
                                    htg[:, k,
                                        (t0 + t) * BT:(t0 + t + 1) * BT],
                                    start=(k == 0), stop=(k == KC1 - 1),
                                )
                                if t > 0:
                                    mi.ins.ldweights = False
                        for t in range(width):
                            nc.scalar.activation(
                                hid[:, m, (t0 + t) * BT:(t0 + t + 1) * BT],
                                pss[t][:],
                                mybir.ActivationFunctionType.Gelu,
                                bias=b1_sb[:, m:m + 1], scale=1.0,
                            )

                # --- L2 (transposed): feT[dd] = W2[:, dd-chunk].T @ hid ---
                # RS granularity: one ReduceScatter per dd-PAIR ([2*P, TG]
                # bf16 = 1 MB input) — small enough to overlap, few enough
                # that per-op rendezvous cost stays well under compute.
                # The very last pair is split into two single-dd chunks so
                # the drain tail after the final matmuls is halved.
                chunks = _rs_chunks(ndd_rs)[g]
                dd2chunk = {}
                for ci, (dd0, ndds, row0) in enumerate(chunks):
                    for dd_ in range(dd0, dd0 + ndds):
                        dd2chunk[dd_] = (ci, dd0, ndds, row0)
                fe_dram = None
                for dd in range(NDD):
                    ci, dd0, ndds, row0 = dd2chunk[dd]
                    if dd == dd0 and use_rs:
                        fe_dram = dram_pool.tile(
                            [ndds, P, TG], fe_dt, tag="fe_dram",
                            name=f"fe_{g}_{ci}")
                    pss = [
                        l1_psum.tile([P, BT], F32, tag="l1",
                                     name=f"l2_{g}_{dd}_{t}")
                        for t in range(NTS)
                    ]
                    for hc in range(MC1):
                        stat = w2_sb[hc][:, dd * P:(dd + 1) * P]
                        for t in range(NTS):
                            mi = nc.tensor.matmul(
                                pss[t][:], stat,
                                hid[:, hc, t * BT:(t + 1) * BT],
                                start=(hc == 0), stop=(hc == MC1 - 1),
                            )
                            if t > 0:
                                mi.ins.ldweights = False
                    for t in range(NTS):
                        fe_sb = fe_pool.tile(
                            [P, BT], fe_dt, tag="fe_sb",
                            name=f"fe_sb_{g}_{dd}_{t}")
                        nc.scalar.activation(
                            fe_sb[:], pss[t][:],
                            mybir.ActivationFunctionType.Identity,
                            bias=b2_sb[:, dd:dd + 1], scale=1.0,
                        )
                        if use_rs:
                            nc.sync.dma_start(
                                fe_dram[dd - dd0, :, t * BT:(t + 1) * BT],
                                fe_sb[:])
                        else:
                            r0 = (g * NDD + dd) * P
                            nc.sync.dma_start(
                                out[r0:r0 + P, t * BT:(t + 1) * BT], fe_sb[:])
                    if dd == dd0 + ndds - 1 and use_rs:
                        rpc = ndds * P // NCORES
                        rs_out = dram_pool.tile(
                            [rpc, TG], fe_dt, tag="rs_dram",
                            name=f"rs_{g}_{ci}")
                        nc.gpsimd.collective_compute(
                            "ReduceScatter",
                            mybir.AluOpType.add,
                            replica_groups=[list(range(NCORES))],
                            ins=[fe_dram[:]],
                            outs=[rs_out[:]],
                        )
                        nc.sync.dma_start(
                            out[row0:row0 + rpc, :], rs_out[:])

    nc.finalize()
    return nc


def _prepare_in_maps_v2(inputs):
    h = np.ascontiguousarray(np.asarray(inputs["h"], dtype=np.float32))
    hT16 = np.ascontiguousarray(h.T.astype(np.float16))  # [IN, B]
    gate_logits = np.asarray(inputs["gate_logits"], dtype=np.float64)
    W1 = np.asarray(inputs["W1"], dtype=np.float32)
    b1 = np.asarray(inputs["b1"], dtype=np.float32)
    W2 = np.asarray(inputs["W2"], dtype=np.float32)
    b2 = np.asarray(inputs["b2"], dtype=np.float32)

    z = np.exp(gate_logits - gate_logits.max())
    probs = (z / z.sum()).astype(np.float32)

    in_maps = []
    for e in range(NCORES):
        w1_e = np.ascontiguousarray(W1[e].astype(np.float16))       # [IN, H]
        b1t_e = np.ascontiguousarray(b1[e].reshape(MC1, P).T)       # [P, MC1]
        w2_e = np.ascontiguousarray(
            (W2[e] * probs[e]).astype(np.float16))                  # [H, D]
        b2t_e = np.ascontiguousarray(
            (b2[e] * probs[e]).reshape(NDD, P).T)                   # [P, NDD]
        in_maps.append(
            {"ht": hT16, "w1": w1_e, "b1t": b1t_e,
             "w2": w2_e, "b2t": b2t_e}
        )
    return in_maps


def _reassemble_v2(results, ndd_rs=2):
    # RS chunk (g, ci) covers dds [dd0, dd0+ndds) as a flat [ndds*P, TG]
    # buffer; core r receives rows r*rpc..(r+1)*rpc of it.
    final = np.empty((B, D), dtype=np.float32)
    sched = _rs_chunks(ndd_rs)
    for r in range(NCORES):
        o = np.asarray(results[r]["out"], dtype=np.float32)  # [256, TG]
        for g in range(NG):
            for dd0, ndds, row0 in sched[g]:
                rpc = ndds * P // NCORES
                blk = o[row0:row0 + rpc, :]                  # [rpc, TG]
                dd = dd0 + (r * rpc) // P
                d0 = dd * P + (r * rpc) % P
                final[g * TG:(g + 1) * TG, d0:d0 + rpc] = blk.T
    return final


# Best measured config: bf16 ReduceScatter (fp16 collectives hit a slow
# path; bf16 runs at full rate and halves the bytes), one RS per
# dd-pair (8 ops of 1 MB). Adds ~5e-3 rel err from bf16 partial-sum
# rounding -- well under the 2e-2 budget.
BEST_CFG = dict(fe_dt=mybir.dt.bfloat16, use_rs=True, ndd_rs=2,
                shared_psum=True)


def _run_v2(inputs, trace=False):
    from concourse.bass_utils import run_bass_kernel_spmd

    in_maps = _prepare_in_maps_v2(inputs)
    nc = _get_nc("v2")
    res = run_bass_kernel_spmd(nc, in_maps, list(range(NCORES)), trace=trace)
    final = _reassemble_v2(res.results, ndd_rs=BEST_CFG["ndd_rs"])
    return final, res


def _get_nc(mm_dtype_name):
    key = mm_dtype_name
    if key not in _CACHE:
        if key == "v2":
            _CACHE[key] = build_v2(**BEST_CFG)
        else:
            _CACHE[key] = build(mm_dtype_name)
    return _CACHE[key]


def _prepare_in_maps(inputs, mm_dtype_name="float16"):
    import ml_dtypes

    np_mm = {"bfloat16": ml_dtypes.bfloat16, "float16": np.float16}.get(
        mm_dtype_name, np.float32
    )
    bf16 = np_mm != np.float32
    h = np.ascontiguousarray(np.asarray(inputs["h"], dtype=np.float32))
    if bf16:
        h = np.ascontiguousarray(h.T.astype(np_mm))  # pre-transposed [IN, B]
    gate_logits = np.asarray(inputs["gate_logits"], dtype=np.float64)
    W1 = np.asarray(inputs["W1"], dtype=np.float32)
    b1 = np.asarray(inputs["b1"], dtype=np.float32)
    W2 = np.asarray(inputs["W2"], dtype=np.float32)
    b2 = np.asarray(inputs["b2"], dtype=np.float32)

    # gate: softmax over E (uniform for zero logits); fold into W2/b2 per expert
    z = np.exp(gate_logits - gate_logits.max())
    probs = (z / z.sum()).astype(np.float32)

    in_maps = []
    for e in range(NCORES):
        w1_e = np.ascontiguousarray(W1[e].astype(np_mm))         # [IN, H]
        b1t_e = np.ascontiguousarray(b1[e].reshape(MC1, P).T)    # [P, MC1]
        w2_e = np.ascontiguousarray((W2[e] * probs[e]).astype(np_mm))  # [H, D]
        b2b_e = np.ascontiguousarray(
            np.broadcast_to(b2[e] * probs[e], (P, D))
        )
        in_maps.append(
            {("ht" if bf16 else "h"): h, "w1": w1_e, "b1t": b1t_e,
             "w2": w2_e, "b2b": b2b_e}
        )
    return in_maps


def _reassemble(results):
    # Reassemble. Chunks: tiles 0..NBT-2 are one 512-row RS each (64 rows per
    # core); the final tile is two 256-row RS (32 rows per core). Core r's
    # shard of a chunk starting at global row g0 with rows_per_core rpc lands
    # at final[g0 + r*rpc : g0 + (r+1)*rpc].
    chunks = []          # (global_row0, out_row0, rows_per_core)
    out_pos = 0
    for t in range(NBT):
        nhalves = 2 if t == NBT - 1 else 1
        rows = BT // nhalves
        for half in range(nhalves):
            rpc = rows // NCORES
            chunks.append((t * BT + half * rows, out_pos, rpc))
            out_pos += rpc
    final = np.empty((B, D), dtype=np.float32)
    for r in range(NCORES):
        o = results[r]["out"]
        for g0, o0, rpc in chunks:
            final[g0 + r * rpc: g0 + (r + 1) * rpc] = o[o0: o0 + rpc]
    return final


def _run(inputs, mm_dtype_name="float16", trace=False):
    from concourse.bass_utils import run_bass_kernel_spmd

    in_maps = _prepare_in_maps(inputs, mm_dtype_name)
    nc = _get_nc(mm_dtype_name)
    res = run_bass_kernel_spmd(nc, in_maps, list(range(NCORES)), trace=trace)
    final = _reassemble(res.results)
    return final, res


def kernel(**inputs):
    mm_dtype_name = os.environ.get("MOE_MM_DTYPE", "v2")
    if mm_dtype_name == "v2":
        final, _ = _run_v2(inputs, trace=False)
    else:
        final, _ = _run(inputs, mm_dtype_name=mm_dtype_name, trace=False)
    return final



# revision 24
# speedup vs baseline: 1.0546x; 1.0546x over previous
# Dense-MoE (all experts active) Trainium2 kernel, expert-parallel over 8
# NeuronCores. Each core computes its expert's 2-layer MLP over all tokens:
#   fe_e = gelu(h @ W1[e] + b1[e]) @ (probs[e] * W2[e]) + probs[e] * b2[e]
# then chunked ReduceScatter(add) across the 8 cores sums the expert
# contributions; the host reassembles the full [B, D] output.
#
# Default path (build_v2): fp16 matmuls, activations transposed on-chip
# (hT [IN, tokens] pre-transposed on host). Two token groups of 2048:
#   L1: for each H-chunk m, one stationary W1 block serves a 4-wide
#       token-tile PSUM sweep (amortizes ldweights 4x); gelu+b1 fused in
#       the ACT read.
#   L2: operands swapped vs the obvious layout -- stationary = W2 block,
#       psum partition = output-D chunk -- so b2 rides the ACT bias port
#       and the per-dd output is produced transposed (feT), again with a
#       4-wide token sweep per stationary.
#   Output: bf16 ReduceScatter per dd-pair (8 ops x 1 MB). fp16
#   collectives hit a ~650 us slow path on this runtime; bf16 runs at
#   full rate. f32 via BEST_CFG["fe_dt"]=F32 if more margin is wanted.
# Measured (chained-dispatch slope minus 224 us fixed overhead):
#   v1 (original): ~609 us;  v2: ~475-505 us;  rel err 5.4e-3 (<2e-2).
# Legacy v1 path kept under MOE_MM_DTYPE=float16/float32r.
import os
import sys

sys.path.insert(0, "/opt/trn_rl_repo")

import numpy as np

import concourse.mybir as mybir
from concourse import bacc, tile
from concourse import masks

B, E, IN, H, D = 4096, 8, 1024, 2048, 1024
NCORES = 8
P = 128
BT = 512                  # tokens per B-tile
NBT = B // BT             # 8 B-tiles
NSUB = BT // P            # 4 token sub-tiles per B-tile
KC1 = IN // P             # 8 contraction chunks, layer 1
MC1 = H // P              # 16 H chunks
ND = D // 512             # 2 output column slices of 512
HALF = 2 * P              # 256 rows per ReduceScatter chunk (1 MB)
RS_ROWS = HALF // NCORES  # 32 rows each core receives per RS chunk
NCHUNK = NBT * 2          # 16 RS chunks

F32 = mybir.dt.float32

_CACHE = {}


def build(mm_dtype_name="float16", nbt=NBT, use_collective=True):
    mm_dt = getattr(mybir.dt, mm_dtype_name)
    bf16 = mybir.dt.size(mm_dt) == 2  # 2-byte path: bf16 or fp16
    nc = bacc.Bacc("TRN2", target_bir_lowering=False)

    if bf16:
        hT = nc.declare_dram_parameter("ht", [IN, nbt * BT], mm_dt, isOutput=False)
    else:
        h = nc.declare_dram_parameter("h", [nbt * BT, IN], F32, isOutput=False)
    w1 = nc.declare_dram_parameter("w1", [IN, H], mm_dt, isOutput=False)
    b1t = nc.declare_dram_parameter("b1t", [P, MC1], F32, isOutput=False)
    w2 = nc.declare_dram_parameter("w2", [H, D], mm_dt, isOutput=False)
    b2b = nc.declare_dram_parameter("b2b", [P, D], F32, isOutput=False)
    out_rows = nbt * BT // NCORES if use_collective else nbt * BT
    out = nc.declare_dram_parameter("out", [out_rows, D], F32, isOutput=True)

    with tile.TileContext(nc) as tc:
        with (
            tc.tile_pool(name="weights", bufs=1) as wpool,
            tc.tile_pool(name="consts", bufs=1) as cpool,
            tc.tile_pool(name="hraw", bufs=2) as hraw_pool,
            tc.tile_pool(name="ht", bufs=(3 if mybir.dt.size(mm_dt) == 2 else 2)) as ht_pool,
            tc.tile_pool(name="hid", bufs=(2 if mybir.dt.size(mm_dt) == 2 else 1)) as hid_pool,
            tc.tile_pool(name="fe", bufs=(2 if mybir.dt.size(mm_dt) == 2 else 1)) as fe_pool,
            tc.tile_pool(name="tp_ps", bufs=(1 if bf16 else 2),
                         space="PSUM") as tp_psum,
            tc.tile_pool(name="l1_ps", bufs=(3 if bf16 else 2),
                         space="PSUM") as l1_psum,
            tc.tile_pool(name="l2_ps", bufs=4, space="PSUM") as l2_psum,
            tc.tile_pool(name="dram", bufs=4, space="DRAM") as dram_pool,
        ):
            hr_pre = []
            ht0 = None
            if bf16:
                ht0 = ht_pool.tile([P, KC1 * BT], mm_dt, tag="ht")
            if not bf16:
                ident = cpool.tile([P, P], F32, tag="ident")
                masks.make_identity(nc, ident[:])

                # prefetch the first two h sub-tiles ahead of the weight slabs
                # so the transpose pipeline starts immediately
                def prefetch_hr(s):
                    hr = hraw_pool.tile([P, IN], F32, tag="hr")
                    nc.sync.dma_start(hr[:], h[s * P:(s + 1) * P, :])
                    hr_pre.append(hr)

                prefetch_hr(0)
                prefetch_hr(1)

            # per-slab weight tiles so the first matmuls depend only on their
            # own slab's DMA; first-tile h loads interleaved so the transpose
            # pipeline is never starved behind 16MB of weights
            w1_sb = []
            for k in range(KC1):
                if bf16:
                    # pair each W1 slab with the matching first-tile hT chunk
                    # so L1's k-accumulation can start as soon as pair 0 lands
                    nc.sync.dma_start(
                        ht0[:, k * BT:(k + 1) * BT],
                        hT[k * P:(k + 1) * P, 0:BT],
                    )
                t_ = wpool.tile([P, H], mm_dt, tag=f"w1_{k}")
                nc.sync.dma_start(t_[:], w1[k * P:(k + 1) * P, :])
                w1_sb.append(t_)
                if k == 3 and not bf16:
                    prefetch_hr(2)
            if not bf16:
                prefetch_hr(3)
            b1_sb = cpool.tile([P, MC1], F32, tag="b1")
            nc.sync.dma_start(b1_sb[:], b1t[:])
            w2_sb = []
            for m in range(MC1):
                t_ = wpool.tile([P, D], mm_dt, tag=f"w2_{m}")
                nc.sync.dma_start(t_[:], w2[m * P:(m + 1) * P, :])
                w2_sb.append(t_)
            b2_sb = cpool.tile([P, D], F32, tag="b2")
            nc.sync.dma_start(b2_sb[:], b2b[:])

            for t in range(nbt):
                # --- transpose this tile's h rows into hT ---
                # ht chunk k (IN rows k*128..) lives at columns [k*BT, (k+1)*BT)
                if bf16 and t == 0:
                    ht = ht0
                else:
                    ht = ht_pool.tile([P, KC1 * BT], mm_dt, tag="ht")
                if bf16 and t > 0:
                    # hT comes pre-transposed from the host: plain strided DMA
                    for k in range(KC1):
                        nc.sync.dma_start(
                            ht[:, k * BT:(k + 1) * BT],
                            hT[k * P:(k + 1) * P, t * BT:(t + 1) * BT],
                        )
                elif not bf16:
                    for s in range(NSUB):
                        if t == 0:
                            hr = hr_pre[s]
                        else:
                            hr = hraw_pool.tile([P, IN], F32, tag="hr")
                            nc.sync.dma_start(
                                hr[:], h[t * BT + s * P: t * BT + (s + 1) * P, :]
                            )
                        for k in range(KC1):
                            tp = tp_psum.tile([P, P], F32, tag="tp")
                            nc.tensor.transpose(
                                tp[:], hr[:, k * P:(k + 1) * P], ident[:]
                            )
                            nc.vector.tensor_copy(
                                ht[:, k * BT + s * P: k * BT + (s + 1) * P], tp[:]
                            )

                # --- layer 1: hidT chunk m = (W1 block).T @ hT, + b1, gelu ---
                # hid is split in two halves so layer 2's sweep releases the
                # first half early for the next tile's evictions
                hidA = hid_pool.tile([P, (MC1 // 2) * BT], mm_dt, tag="hidA")
                hidB = hid_pool.tile([P, (MC1 // 2) * BT], mm_dt, tag="hidB")

                def hid_slice(m, lo, hi):
                    half_t = hidA if m < MC1 // 2 else hidB
                    mm_ = m % (MC1 // 2)
                    return half_t[:, mm_ * BT + lo: mm_ * BT + hi]

                for m in range(MC1):
                    ps = l1_psum.tile([P, BT], F32, tag="l1")
                    for k in range(KC1):
                        nc.tensor.matmul(
                            ps[:],
                            w1_sb[k][:, m * P:(m + 1) * P],
                            ht[:, k * BT:(k + 1) * BT],
                            start=(k == 0),
                            stop=(k == KC1 - 1),
                        )
                    nc.scalar.activation(
                        hid_slice(m, 0, BT),
                        ps[:],
                        mybir.ActivationFunctionType.Gelu,
                        bias=b1_sb[:, m:m + 1],
                        scale=1.0,
                    )

                # --- layer 2 + chunked ReduceScatter (2MB per tile,
                # tapering to 2x1MB on the final tile for a short tail) ---
                nhalves = 2 if t == nbt - 1 else 1
                subs_per_chunk = NSUB // nhalves
                for half in range(nhalves):
                    fe_chunk = dram_pool.tile(
                        [subs_per_chunk * P, D], F32, tag="fe_dram"
                    )
                    for si in range(subs_per_chunk):
                        s = half * subs_per_chunk + si
                        # both d-slices accumulate together: the second matmul
                        # of each pair reuses the stationary hid block already
                        # in the PE array (ldweights=False) instead of
                        # reloading it
                        ps_a = l2_psum.tile([P, 512], F32, tag="l2")
                        ps_b = l2_psum.tile([P, 512], F32, tag="l2")
                        pss = [ps_a, ps_b]
                        for m in range(MC1):
                            hs = hid_slice(m, s * P, (s + 1) * P)
                            for d in range(ND):
                                mi = nc.tensor.matmul(
                                    pss[d][:],
                                    hs,
                                    w2_sb[m][:, d * 512:(d + 1) * 512],
                                    start=(m == 0),
                                    stop=(m == MC1 - 1),
                                )
                                if d > 0:
                                    mi.ins.ldweights = False
                        for d in range(ND):
                            fe_sb = fe_pool.tile([P, 512], F32, tag="fe_sb")
                            nc.vector.tensor_add(
                                fe_sb[:], pss[d][:],
                                b2_sb[:, d * 512:(d + 1) * 512]
                            )
                            nc.sync.dma_start(
                                fe_chunk[si * P:(si + 1) * P,
                                         d * 512:(d + 1) * 512],
                                fe_sb[:],
                            )

                    chunk_rows = subs_per_chunk * P // NCORES
                    row0 = (t * BT + half * subs_per_chunk * P) // NCORES
                    if use_collective:
                        rs_chunk = dram_pool.tile(
                            [chunk_rows, D], F32, tag="rs_dram"
                        )
                        nc.gpsimd.collective_compute(
                            "ReduceScatter",
                            mybir.AluOpType.add,
                            replica_groups=[list(range(NCORES))],
                            ins=[fe_chunk[:]],
                            outs=[rs_chunk[:]],
                        )
                        nc.sync.dma_start(
                            out[row0:row0 + chunk_rows, :], rs_chunk[:]
                        )
                    else:
                        r0 = t * BT + half * subs_per_chunk * P
                        nc.sync.dma_start(
                            out[r0:r0 + subs_per_chunk * P, :], fe_chunk[:]
                        )

    nc.finalize()
    return nc


TG = 2048                 # tokens per group (v2)
NG = B // TG              # 2 groups
NTS = TG // BT            # 4 token sub-tiles of 512 per group
NDD = D // P              # 8 output d-chunks of 128
F16 = mybir.dt.float16


def _rs_chunks(ndd_rs):
    """Per-group RS chunk schedule: list (per g) of (dd0, ndds, out_row0).
    Integer ndd_rs: fixed-size chunks, with the final group's last chunk
    split into singles to shorten the drain tail. "taper": front-loaded
    quads early (transfer volume overlaps remaining compute), singles at
    the very end (minimal tail), fewest rendezvous (6 ops)."""
    if ndd_rs == "taper":
        sizes = {g: ([4, 4] if g < NG - 1 else [4, 2, 1, 1])
                 for g in range(NG)}
    else:
        sizes = {}
        for g in range(NG):
            ss, dd = [], 0
            while dd < NDD:
                n = ndd_rs
                if g == NG - 1 and dd >= NDD - ndd_rs and ndd_rs > 1:
                    n = 1
                ss.append(n)
                dd += n
            sizes[g] = ss
    sched = []
    row = 0
    for g in range(NG):
        chunks, dd = [], 0
        for n in sizes[g]:
            chunks.append((dd, n, row))
            row += n * P // NCORES
            dd += n
        sched.append(chunks)
    return sched


def build_v2(fe_dt=F16, use_rs=True, ndd_rs=2, shared_psum=False):
    """fp16 kernel, v2: stationary-reuse token sweeps in both layers,
    L2 emits transposed output (psum partition = d) so the b2 bias rides
    the ACT instruction, ReduceScatter per dd-pair."""
    nc = bacc.Bacc("TRN2", target_bir_lowering=False)

    hT = nc.declare_dram_parameter("ht", [IN, B], F16, isOutput=False)
    w1 = nc.declare_dram_parameter("w1", [IN, H], F16, isOutput=False)
    b1t = nc.declare_dram_parameter("b1t", [P, MC1], F32, isOutput=False)
    w2 = nc.declare_dram_parameter("w2", [H, D], F16, isOutput=False)
    b2t = nc.declare_dram_parameter("b2t", [P, NDD], F32, isOutput=False)
    # out rows: (g, dd) chunk -> 16 d-rows per core, 2048 tokens
    out_rows = NG * NDD * (P // NCORES) if use_rs else NG * NDD * P
    out = nc.declare_dram_parameter("out", [out_rows, TG], fe_dt,
                                    isOutput=True)

    with tile.TileContext(nc) as tc:
        with (
            tc.tile_pool(name="weights", bufs=1) as wpool,
            tc.tile_pool(name="consts", bufs=1) as cpool,
            tc.tile_pool(name="ht", bufs=2) as ht_pool,
            tc.tile_pool(name="hid", bufs=1) as hid_pool,
            tc.tile_pool(name="fe", bufs=4) as fe_pool,
            tc.tile_pool(name="l1_ps", bufs=(8 if shared_psum else 4),
                         space="PSUM") as l1_psum,
            tc.tile_pool(name="dram", bufs=6, space="DRAM") as dram_pool,
        ):
            # --- weight / bias / first-group hT loads (interleaved) ---
            # Startup staging: the first m-chunks of L1 run as 2-wide
            # half-sweeps over tokens 0:TG/2, so the critical path to the
            # first matmuls is one hT token-half (2 MB) + one w1 column
            # quarter (1 MB) instead of the full 8 MB.
            ht0 = ht_pool.tile([P, KC1, TG], F16, tag="ht", name="ht_g0")
            w1_sb = []
            for k in range(KC1):
                nc.sync.dma_start(
                    ht0[:, k, 0:TG // 2], hT[k * P:(k + 1) * P, 0:TG // 2])
                t_ = wpool.tile([P, H], F16, tag=f"w1_{k}", name=f"w1_{k}")
                nc.sync.dma_start(
                    t_[:, 0:H // 4], w1[k * P:(k + 1) * P, 0:H // 4])
                w1_sb.append(t_)
            b1_sb = cpool.tile([P, MC1], F32, tag="b1", name="b1_sb")
            nc.sync.dma_start(b1_sb[:], b1t[:])
            for k in range(KC1):
                nc.sync.dma_start(
                    ht0[:, k, TG // 2:TG],
                    hT[k * P:(k + 1) * P, TG // 2:TG])
                nc.sync.dma_start(
                    w1_sb[k][:, H // 4:H // 2],
                    w1[k * P:(k + 1) * P, H // 4:H // 2])
            for k in range(KC1):
                nc.sync.dma_start(
                    w1_sb[k][:, H // 2:H],
                    w1[k * P:(k + 1) * P, H // 2:H])
            w2_sb = []
            for m in range(MC1):
                t_ = wpool.tile([P, D], F16, tag=f"w2_{m}", name=f"w2_{m}")
                nc.sync.dma_start(t_[:], w2[m * P:(m + 1) * P, :])
                w2_sb.append(t_)
            b2_sb = cpool.tile([P, NDD], F32, tag="b2", name="b2_sb")
            nc.sync.dma_start(b2_sb[:], b2t[:])

            ht_tiles = [ht0]
            for g in range(1, NG):
                htg = ht_pool.tile([P, KC1, TG], F16, tag="ht", name=f"ht_g{g}")
                for k in range(KC1):
                    nc.sync.dma_start(
                        htg[:, k], hT[k * P:(k + 1) * P, g * TG:(g + 1) * TG])
                ht_tiles.append(htg)

            for g in range(NG):
                htg = ht_tiles[g]
                hid = hid_pool.tile([P, MC1, TG], F16, tag="hid",
                                    name=f"hid_g{g}")

                # --- L1: hid[m, :] = gelu(W1[:,m-chunk].T @ ht + b1) ---
                for m in range(MC1):
                    # group 0's first two m-chunks run as 2-wide
                    # half-sweeps so they only depend on the first hT
                    # token-half (see startup staging above)
                    halves = 2 if (g == 0 and m < 2) else 1
                    width = NTS // halves
                    for hf in range(halves):
                        pss = [
                            l1_psum.tile([P, BT], F32, tag="l1",
                                         name=f"l1_{g}_{m}_{hf}_{t}")
                            for t in range(width)
                        ]
                        t0 = hf * width
                        for k in range(KC1):
                            stat = w1_sb[k][:, m * P:(m + 1) * P]
                            for t in range(width):
                                mi = nc.tensor.matmul(
                                    pss[t][:], stat,
                                    htg[:, k,
                                        (t0 + t) * BT:(t0 + t + 1) * BT],
                                    start=(k == 0), stop=(k == KC1 - 1),
                                )
                                if t > 0:
                                    mi.ins.ldweights = False
                        for t in range(width):
                            nc.scalar.activation(
                                hid[:, m, (t0 + t) * BT:(t0 + t + 1) * BT],
                                pss[t][:],
                                mybir.ActivationFunctionType.Gelu,
                                bias=b1_sb[:, m:m + 1], scale=1.0,
                            )

                # --- L2 (transposed): feT[dd] = W2[:, dd-chunk].T @ hid ---
                # RS granularity: one ReduceScatter per dd-PAIR ([2*P, TG]
                # bf16 = 1 MB input) — small enough to overlap, few enough
                # that per-op rendezvous cost stays well under compute.
                # The very last pair is split into two single-dd chunks so
                # the drain tail after the final matmuls is halved.
                chunks = _rs_chunks(ndd_rs)[g]
                dd2chunk = {}
                for ci, (dd0, ndds, row0) in enumerate(chunks):
                    for dd_ in range(dd0, dd0 + ndds):
                        dd2chunk[dd_] = (ci, dd0, ndds, row0)
                fe_dram = None
                for dd in range(NDD):
                    ci, dd0, ndds, row0 = dd2chunk[dd]
                    if dd == dd0 and use_rs:
                        fe_dram = dram_pool.tile(
                            [ndds, P, TG], fe_dt, tag="fe_dram",
                            name=f"fe_{g}_{ci}")
                    pss = [
                        l1_psum.tile([P, BT], F32, tag="l1",
                                     name=f"l2_{g}_{dd}_{t}")
                        for t in range(NTS)
                    ]
                    for hc in range(MC1):
                        stat = w2_sb[hc][:, dd * P:(dd + 1) * P]
                        for t in range(NTS):
                            mi = nc.tensor.matmul(
                                pss[t][:], stat,
                                hid[:, hc, t * BT:(t + 1) * BT],
                                start=(hc == 0), stop=(hc == MC1 - 1),
                            )
                            if t > 0:
                                mi.ins.ldweights = False
                    for t in range(NTS):
                        fe_sb = fe_pool.tile(
                            [P, BT], fe_dt, tag="fe_sb",
                            name=f"fe_sb_{g}_{dd}_{t}")
                        nc.scalar.activation(
                            fe_sb[:], pss[t][:],
                            mybir.ActivationFunctionType.Identity,
                            bias=b2_sb[:, dd:dd + 1], scale=1.0,
                        )
                        if use_rs:
                            nc.sync.dma_start(
                                fe_dram[dd - dd0, :, t * BT:(t + 1) * BT],
                                fe_sb[:])
                        else:
                            r0 = (g * NDD + dd) * P
                            nc.sync.dma_start(
                                out[r0:r0 + P, t * BT:(t + 1) * BT], fe_sb[:])
                    if dd == dd0 + ndds - 1 and use_rs:
                        rpc = ndds * P // NCORES
                        rs_out = dram_pool.tile(
                            [rpc, TG], fe_dt, tag="rs_dram",
                            name=f"rs_{g}_{ci}")
                        nc.gpsimd.collective_compute(
                            "ReduceScatter",
                            mybir.AluOpType.add,
                            replica_groups=[list(range(NCORES))],
                            ins=[fe_dram[:]],
                            outs=[rs_out[:]],
                        )
                        nc.sync.dma_start(
                            out[row0:row0 + rpc, :], rs_out[:])

    nc.finalize()
    return nc


def _prepare_in_maps_v2(inputs):
    h = np.ascontiguousarray(np.asarray(inputs["h"], dtype=np.float32))
    hT16 = np.ascontiguousarray(h.T.astype(np.float16))  # [IN, B]
    gate_logits = np.asarray(inputs["gate_logits"], dtype=np.float64)
    W1 = np.asarray(inputs["W1"], dtype=np.float32)
    b1 = np.asarray(inputs["b1"], dtype=np.float32)
    W2 = np.asarray(inputs["W2"], dtype=np.float32)
    b2 = np.asarray(inputs["b2"], dtype=np.float32)

    z = np.exp(gate_logits - gate_logits.max())
    probs = (z / z.sum()).astype(np.float32)

    in_maps = []
    for e in range(NCORES):
        w1_e = np.ascontiguousarray(W1[e].astype(np.float16))       # [IN, H]
        b1t_e = np.ascontiguousarray(b1[e].reshape(MC1, P).T)       # [P, MC1]
        w2_e = np.ascontiguousarray(
            (W2[e] * probs[e]).astype(np.float16))                  # [H, D]
        b2t_e = np.ascontiguousarray(
            (b2[e] * probs[e]).reshape(NDD, P).T)                   # [P, NDD]
        in_maps.append(
            {"ht": hT16, "w1": w1_e, "b1t": b1t_e,
             "w2": w2_e, "b2t": b2t_e}
        )
    return in_maps


def _reassemble_v2(results, ndd_rs=2):
    # RS chunk (g, ci) covers dds [dd0, dd0+ndds) as a flat [ndds*P, TG]
    # buffer; core r receives rows r*rpc..(r+1)*rpc of it.
    final = np.empty((B, D), dtype=np.float32)
    sched = _rs_chunks(ndd_rs)
    for r in range(NCORES):
        o = np.asarray(results[r]["out"], dtype=np.float32)  # [256, TG]
        for g in range(NG):
            for dd0, ndds, row0 in sched[g]:
                rpc = ndds * P // NCORES
                blk = o[row0:row0 + rpc, :]                  # [rpc, TG]
                dd = dd0 + (r * rpc) // P
                d0 = dd * P + (r * rpc) % P
                final[g * TG:(g + 1) * TG, d0:d0 + rpc] = blk.T
    return final


# Best measured config: bf16 ReduceScatter (fp16 collectives hit a slow
# path; bf16 runs at full rate and halves the bytes), one RS per
# dd-pair (8 ops of 1 MB). Adds ~5e-3 rel err from bf16 partial-sum
# rounding -- well under the 2e-2 budget.
BEST_CFG = dict(fe_dt=mybir.dt.bfloat16, use_rs=True, ndd_rs="taper",
                shared_psum=True)


def _run_v2(inputs, trace=False):
    from concourse.bass_utils import run_bass_kernel_spmd

    in_maps = _prepare_in_maps_v2(inputs)
    nc = _get_nc("v2")
    res = run_bass_kernel_spmd(nc, in_maps, list(range(NCORES)), trace=trace)
    final = _reassemble_v2(res.results, ndd_rs=BEST_CFG["ndd_rs"])
    return final, res


def _get_nc(mm_dtype_name):
    key = mm_dtype_name
    if key not in _CACHE:
        if key == "v2":
            _CACHE[key] = build_v2(**BEST_CFG)
        else:
            _CACHE[key] = build(mm_dtype_name)
    return _CACHE[key]


def _prepare_in_maps(inputs, mm_dtype_name="float16"):
    import ml_dtypes

    np_mm = {"bfloat16": ml_dtypes.bfloat16, "float16": np.float16}.get(
        mm_dtype_name, np.float32
    )
    bf16 = np_mm != np.float32
    h = np.ascontiguousarray(np.asarray(inputs["h"], dtype=np.float32))
    if bf16:
        h = np.ascontiguousarray(h.T.astype(np_mm))  # pre-transposed [IN, B]
    gate_logits = np.asarray(inputs["gate_logits"], dtype=np.float64)
    W1 = np.asarray(inputs["W1"], dtype=np.float32)
    b1 = np.asarray(inputs["b1"], dtype=np.float32)
    W2 = np.asarray(inputs["W2"], dtype=np.float32)
    b2 = np.asarray(inputs["b2"], dtype=np.float32)

    # gate: softmax over E (uniform for zero logits); fold into W2/b2 per expert
    z = np.exp(gate_logits - gate_logits.max())
    probs = (z / z.sum()).astype(np.float32)

    in_maps = []
    for e in range(NCORES):
        w1_e = np.ascontiguousarray(W1[e].astype(np_mm))         # [IN, H]
        b1t_e = np.ascontiguousarray(b1[e].reshape(MC1, P).T)    # [P, MC1]
        w2_e = np.ascontiguousarray((W2[e] * probs[e]).astype(np_mm))  # [H, D]
        b2b_e = np.ascontiguousarray(
            np.broadcast_to(b2[e] * probs[e], (P, D))
        )
        in_maps.append(
            {("ht" if bf16 else "h"): h, "w1": w1_e, "b1t": b1t_e,
             "w2": w2_e, "b2b": b2b_e}
        )
    return in_maps


def _reassemble(results):
    # Reassemble. Chunks: tiles 0..NBT-2 are one 512-row RS each (64 rows per
    # core); the final tile is two 256-row RS (32 rows per core). Core r's
    # shard of a chunk starting at global row g0 with rows_per_core rpc lands
    # at final[g0 + r*rpc : g0 + (r+1)*rpc].
    chunks = []          # (global_row0, out_row0, rows_per_core)
    out_pos = 0
    for t in range(NBT):
        nhalves = 2 if t == NBT - 1 else 1
        rows = BT // nhalves
        for half in range(nhalves):
            rpc = rows // NCORES
            chunks.append((t * BT + half * rows, out_pos, rpc))
            out_pos += rpc
    final = np.empty((B, D), dtype=np.float32)
    for r in range(NCORES):
        o = results[r]["out"]
        for g0, o0, rpc in chunks:
            final[g0 + r * rpc: g0 + (r + 1) * rpc] = o[o0: o0 + rpc]
    return final


def _run(inputs, mm_dtype_name="float16", trace=False):
    from concourse.bass_utils import run_bass_kernel_spmd

    in_maps = _prepare_in_maps(inputs, mm_dtype_name)
    nc = _get_nc(mm_dtype_name)
    res = run_bass_kernel_spmd(nc, in_maps, list(range(NCORES)), trace=trace)
    final = _reassemble(res.results)
    return final, res


def kernel(**inputs):
    mm_dtype_name = os.environ.get("MOE_MM_DTYPE", "v2")
    if mm_dtype_name == "v2":
        final, _ = _run_v2(inputs, trace=False)
    else:
        final, _ = _run(inputs, mm_dtype_name=mm_dtype_name, trace=False)
    return final



# revision 26
# speedup vs baseline: 1.1182x; 1.0603x over previous
# Dense-MoE (all experts active) Trainium2 kernel, expert-parallel over 8
# NeuronCores. Each core computes its expert's 2-layer MLP over all tokens:
#   fe_e = gelu(h @ W1[e] + b1[e]) @ (probs[e] * W2[e]) + probs[e] * b2[e]
# then chunked ReduceScatter(add) across the 8 cores sums the expert
# contributions; the host reassembles the full [B, D] output.
#
# Default path (build_v2): fp16 matmuls, activations transposed on-chip
# (hT [IN, tokens] pre-transposed on host). Two token groups of 2048:
#   L1: for each H-chunk m, one stationary W1 block serves a 4-wide
#       token-tile PSUM sweep (amortizes ldweights 4x); gelu+b1 fused in
#       the ACT read.
#   L2: operands swapped vs the obvious layout -- stationary = W2 block,
#       psum partition = output-D chunk -- so b2 rides the ACT bias port
#       and the per-dd output is produced transposed (feT), again with a
#       4-wide token sweep per stationary.
#   Output: bf16 ReduceScatter on a tapered schedule (quads early so
#   transfer volume overlaps remaining compute, singles last for a
#   minimal drain tail; 6 ops total). fp16 collectives hit a ~650 us
#   slow path on this runtime; bf16 runs at full rate. f32 via
#   BEST_CFG["fe_dt"]=F32 if more margin is wanted.
# Measured (chained-dispatch slope minus 224 us fixed overhead):
#   v1 (original): ~609 us;  v2: ~475-505 us;  rel err 5.4e-3 (<2e-2).
# Legacy v1 path kept under MOE_MM_DTYPE=float16/float32r.
import os
import sys

sys.path.insert(0, "/opt/trn_rl_repo")

import numpy as np

import concourse.mybir as mybir
from concourse import bacc, tile
from concourse import masks

B, E, IN, H, D = 4096, 8, 1024, 2048, 1024
NCORES = 8
P = 128
BT = 512                  # tokens per B-tile
NBT = B // BT             # 8 B-tiles
NSUB = BT // P            # 4 token sub-tiles per B-tile
KC1 = IN // P             # 8 contraction chunks, layer 1
MC1 = H // P              # 16 H chunks
ND = D // 512             # 2 output column slices of 512
HALF = 2 * P              # 256 rows per ReduceScatter chunk (1 MB)
RS_ROWS = HALF // NCORES  # 32 rows each core receives per RS chunk
NCHUNK = NBT * 2          # 16 RS chunks

F32 = mybir.dt.float32

_CACHE = {}


def build(mm_dtype_name="float16", nbt=NBT, use_collective=True):
    mm_dt = getattr(mybir.dt, mm_dtype_name)
    bf16 = mybir.dt.size(mm_dt) == 2  # 2-byte path: bf16 or fp16
    nc = bacc.Bacc("TRN2", target_bir_lowering=False)

    if bf16:
        hT = nc.declare_dram_parameter("ht", [IN, nbt * BT], mm_dt, isOutput=False)
    else:
        h = nc.declare_dram_parameter("h", [nbt * BT, IN], F32, isOutput=False)
    w1 = nc.declare_dram_parameter("w1", [IN, H], mm_dt, isOutput=False)
    b1t = nc.declare_dram_parameter("b1t", [P, MC1], F32, isOutput=False)
    w2 = nc.declare_dram_parameter("w2", [H, D], mm_dt, isOutput=False)
    b2b = nc.declare_dram_parameter("b2b", [P, D], F32, isOutput=False)
    out_rows = nbt * BT // NCORES if use_collective else nbt * BT
    out = nc.declare_dram_parameter("out", [out_rows, D], F32, isOutput=True)

    with tile.TileContext(nc) as tc:
        with (
            tc.tile_pool(name="weights", bufs=1) as wpool,
            tc.tile_pool(name="consts", bufs=1) as cpool,
            tc.tile_pool(name="hraw", bufs=2) as hraw_pool,
            tc.tile_pool(name="ht", bufs=(3 if mybir.dt.size(mm_dt) == 2 else 2)) as ht_pool,
            tc.tile_pool(name="hid", bufs=(2 if mybir.dt.size(mm_dt) == 2 else 1)) as hid_pool,
            tc.tile_pool(name="fe", bufs=(2 if mybir.dt.size(mm_dt) == 2 else 1)) as fe_pool,
            tc.tile_pool(name="tp_ps", bufs=(1 if bf16 else 2),
                         space="PSUM") as tp_psum,
            tc.tile_pool(name="l1_ps", bufs=(3 if bf16 else 2),
                         space="PSUM") as l1_psum,
            tc.tile_pool(name="l2_ps", bufs=4, space="PSUM") as l2_psum,
            tc.tile_pool(name="dram", bufs=4, space="DRAM") as dram_pool,
        ):
            hr_pre = []
            ht0 = None
            if bf16:
                ht0 = ht_pool.tile([P, KC1 * BT], mm_dt, tag="ht")
            if not bf16:
                ident = cpool.tile([P, P], F32, tag="ident")
                masks.make_identity(nc, ident[:])

                # prefetch the first two h sub-tiles ahead of the weight slabs
                # so the transpose pipeline starts immediately
                def prefetch_hr(s):
                    hr = hraw_pool.tile([P, IN], F32, tag="hr")
                    nc.sync.dma_start(hr[:], h[s * P:(s + 1) * P, :])
                    hr_pre.append(hr)

                prefetch_hr(0)
                prefetch_hr(1)

            # per-slab weight tiles so the first matmuls depend only on their
            # own slab's DMA; first-tile h loads interleaved so the transpose
            # pipeline is never starved behind 16MB of weights
            w1_sb = []
            for k in range(KC1):
                if bf16:
                    # pair each W1 slab with the matching first-tile hT chunk
                    # so L1's k-accumulation can start as soon as pair 0 lands
                    nc.sync.dma_start(
                        ht0[:, k * BT:(k + 1) * BT],
                        hT[k * P:(k + 1) * P, 0:BT],
                    )
                t_ = wpool.tile([P, H], mm_dt, tag=f"w1_{k}")
                nc.sync.dma_start(t_[:], w1[k * P:(k + 1) * P, :])
                w1_sb.append(t_)
                if k == 3 and not bf16:
                    prefetch_hr(2)
            if not bf16:
                prefetch_hr(3)
            b1_sb = cpool.tile([P, MC1], F32, tag="b1")
            nc.sync.dma_start(b1_sb[:], b1t[:])
            w2_sb = []
            for m in range(MC1):
                t_ = wpool.tile([P, D], mm_dt, tag=f"w2_{m}")
                nc.sync.dma_start(t_[:], w2[m * P:(m + 1) * P, :])
                w2_sb.append(t_)
            b2_sb = cpool.tile([P, D], F32, tag="b2")
            nc.sync.dma_start(b2_sb[:], b2b[:])

            for t in range(nbt):
                # --- transpose this tile's h rows into hT ---
                # ht chunk k (IN rows k*128..) lives at columns [k*BT, (k+1)*BT)
                if bf16 and t == 0:
                    ht = ht0
                else:
                    ht = ht_pool.tile([P, KC1 * BT], mm_dt, tag="ht")
                if bf16 and t > 0:
                    # hT comes pre-transposed from the host: plain strided DMA
                    for k in range(KC1):
                        nc.sync.dma_start(
                            ht[:, k * BT:(k + 1) * BT],
                            hT[k * P:(k + 1) * P, t * BT:(t + 1) * BT],
                        )
                elif not bf16:
                    for s in range(NSUB):
                        if t == 0:
                            hr = hr_pre[s]
                        else:
                            hr = hraw_pool.tile([P, IN], F32, tag="hr")
                            nc.sync.dma_start(
                                hr[:], h[t * BT + s * P: t * BT + (s + 1) * P, :]
                            )
                        for k in range(KC1):
                            tp = tp_psum.tile([P, P], F32, tag="tp")
                            nc.tensor.transpose(
                                tp[:], hr[:, k * P:(k + 1) * P], ident[:]
                            )
                            nc.vector.tensor_copy(
                                ht[:, k * BT + s * P: k * BT + (s + 1) * P], tp[:]
                            )

                # --- layer 1: hidT chunk m = (W1 block).T @ hT, + b1, gelu ---
                # hid is split in two halves so layer 2's sweep releases the
                # first half early for the next tile's evictions
                hidA = hid_pool.tile([P, (MC1 // 2) * BT], mm_dt, tag="hidA")
                hidB = hid_pool.tile([P, (MC1 // 2) * BT], mm_dt, tag="hidB")

                def hid_slice(m, lo, hi):
                    half_t = hidA if m < MC1 // 2 else hidB
                    mm_ = m % (MC1 // 2)
                    return half_t[:, mm_ * BT + lo: mm_ * BT + hi]

                for m in range(MC1):
                    ps = l1_psum.tile([P, BT], F32, tag="l1")
                    for k in range(KC1):
                        nc.tensor.matmul(
                            ps[:],
                            w1_sb[k][:, m * P:(m + 1) * P],
                            ht[:, k * BT:(k + 1) * BT],
                            start=(k == 0),
                            stop=(k == KC1 - 1),
                        )
                    nc.scalar.activation(
                        hid_slice(m, 0, BT),
                        ps[:],
                        mybir.ActivationFunctionType.Gelu,
                        bias=b1_sb[:, m:m + 1],
                        scale=1.0,
                    )

                # --- layer 2 + chunked ReduceScatter (2MB per tile,
                # tapering to 2x1MB on the final tile for a short tail) ---
                nhalves = 2 if t == nbt - 1 else 1
                subs_per_chunk = NSUB // nhalves
                for half in range(nhalves):
                    fe_chunk = dram_pool.tile(
                        [subs_per_chunk * P, D], F32, tag="fe_dram"
                    )
                    for si in range(subs_per_chunk):
                        s = half * subs_per_chunk + si
                        # both d-slices accumulate together: the second matmul
                        # of each pair reuses the stationary hid block already
                        # in the PE array (ldweights=False) instead of
                        # reloading it
                        ps_a = l2_psum.tile([P, 512], F32, tag="l2")
                        ps_b = l2_psum.tile([P, 512], F32, tag="l2")
                        pss = [ps_a, ps_b]
                        for m in range(MC1):
                            hs = hid_slice(m, s * P, (s + 1) * P)
                            for d in range(ND):
                                mi = nc.tensor.matmul(
                                    pss[d][:],
                                    hs,
                                    w2_sb[m][:, d * 512:(d + 1) * 512],
                                    start=(m == 0),
                                    stop=(m == MC1 - 1),
                                )
                                if d > 0:
                                    mi.ins.ldweights = False
                        for d in range(ND):
                            fe_sb = fe_pool.tile([P, 512], F32, tag="fe_sb")
                            nc.vector.tensor_add(
                                fe_sb[:], pss[d][:],
                                b2_sb[:, d * 512:(d + 1) * 512]
                            )
                            nc.sync.dma_start(
                                fe_chunk[si * P:(si + 1) * P,
                                         d * 512:(d + 1) * 512],
                                fe_sb[:],
                            )

                    chunk_rows = subs_per_chunk * P // NCORES
                    row0 = (t * BT + half * subs_per_chunk * P) // NCORES
                    if use_collective:
                        rs_chunk = dram_pool.tile(
                            [chunk_rows, D], F32, tag="rs_dram"
                        )
                        nc.gpsimd.collective_compute(
                            "ReduceScatter",
                            mybir.AluOpType.add,
                            replica_groups=[list(range(NCORES))],
                            ins=[fe_chunk[:]],
                            outs=[rs_chunk[:]],
                        )
                        nc.sync.dma_start(
                            out[row0:row0 + chunk_rows, :], rs_chunk[:]
                        )
                    else:
                        r0 = t * BT + half * subs_per_chunk * P
                        nc.sync.dma_start(
                            out[r0:r0 + subs_per_chunk * P, :], fe_chunk[:]
                        )

    nc.finalize()
    return nc


TG = 2048                 # tokens per group (v2)
NG = B // TG              # 2 groups
NTS = TG // BT            # 4 token sub-tiles of 512 per group
NDD = D // P              # 8 output d-chunks of 128
F16 = mybir.dt.float16


def _rs_chunks(ndd_rs):
    """Per-group RS chunk schedule: list (per g) of (dd0, ndds, out_row0).
    Integer ndd_rs: fixed-size chunks, with the final group's last chunk
    split into singles to shorten the drain tail. "taper": front-loaded
    quads early (transfer volume overlaps remaining compute), singles at
    the very end (minimal tail), fewest rendezvous (6 ops)."""
    if ndd_rs == "taper":
        sizes = {g: ([4, 4] if g < NG - 1 else [4, 2, 1, 1])
                 for g in range(NG)}
    else:
        sizes = {}
        for g in range(NG):
            ss, dd = [], 0
            while dd < NDD:
                n = ndd_rs
                if g == NG - 1 and dd >= NDD - ndd_rs and ndd_rs > 1:
                    n = 1
                ss.append(n)
                dd += n
            sizes[g] = ss
    sched = []
    row = 0
    for g in range(NG):
        chunks, dd = [], 0
        for n in sizes[g]:
            chunks.append((dd, n, row))
            row += n * P // NCORES
            dd += n
        sched.append(chunks)
    return sched


def build_v2(fe_dt=F16, use_rs=True, ndd_rs=2, shared_psum=False,
             explicit_ldw=False):
    """fp16 kernel, v2: stationary-reuse token sweeps in both layers,
    L2 emits transposed output (psum partition = d) so the b2 bias rides
    the ACT instruction, ReduceScatter per dd-pair."""
    nc = bacc.Bacc("TRN2", target_bir_lowering=False)

    hT = nc.declare_dram_parameter("ht", [IN, B], F16, isOutput=False)
    w1 = nc.declare_dram_parameter("w1", [IN, H], F16, isOutput=False)
    b1t = nc.declare_dram_parameter("b1t", [P, MC1], F32, isOutput=False)
    w2 = nc.declare_dram_parameter("w2", [H, D], F16, isOutput=False)
    b2t = nc.declare_dram_parameter("b2t", [P, NDD], F32, isOutput=False)
    # out rows: (g, dd) chunk -> 16 d-rows per core, 2048 tokens
    out_rows = NG * NDD * (P // NCORES) if use_rs else NG * NDD * P
    out = nc.declare_dram_parameter("out", [out_rows, TG], fe_dt,
                                    isOutput=True)

    with tile.TileContext(nc) as tc:
        with (
            tc.tile_pool(name="weights", bufs=1) as wpool,
            tc.tile_pool(name="consts", bufs=1) as cpool,
            tc.tile_pool(name="ht", bufs=2) as ht_pool,
            tc.tile_pool(name="hid", bufs=1) as hid_pool,
            tc.tile_pool(name="fe", bufs=4) as fe_pool,
            tc.tile_pool(name="l1_ps", bufs=(8 if shared_psum else 4),
                         space="PSUM") as l1_psum,
            tc.tile_pool(name="dram", bufs=6, space="DRAM") as dram_pool,
        ):
            # --- weight / bias / first-group hT loads (interleaved) ---
            # Startup staging: the first m-chunks of L1 run as 2-wide
            # half-sweeps over tokens 0:TG/2, so the critical path to the
            # first matmuls is one hT token-half (2 MB) + one w1 column
            # quarter (1 MB) instead of the full 8 MB.
            ht0 = ht_pool.tile([P, KC1, TG], F16, tag="ht", name="ht_g0")
            w1_sb = []
            for k in range(KC1):
                nc.sync.dma_start(
                    ht0[:, k, 0:TG // 2], hT[k * P:(k + 1) * P, 0:TG // 2])
                t_ = wpool.tile([P, H], F16, tag=f"w1_{k}", name=f"w1_{k}")
                nc.sync.dma_start(
                    t_[:, 0:H // 4], w1[k * P:(k + 1) * P, 0:H // 4])
                w1_sb.append(t_)
            b1_sb = cpool.tile([P, MC1], F32, tag="b1", name="b1_sb")
            nc.sync.dma_start(b1_sb[:], b1t[:])
            for k in range(KC1):
                nc.sync.dma_start(
                    ht0[:, k, TG // 2:TG],
                    hT[k * P:(k + 1) * P, TG // 2:TG])
                nc.sync.dma_start(
                    w1_sb[k][:, H // 4:H // 2],
                    w1[k * P:(k + 1) * P, H // 4:H // 2])
            for k in range(KC1):
                nc.sync.dma_start(
                    w1_sb[k][:, H // 2:H],
                    w1[k * P:(k + 1) * P, H // 2:H])
            w2_sb = []
            for m in range(MC1):
                t_ = wpool.tile([P, D], F16, tag=f"w2_{m}", name=f"w2_{m}")
                nc.sync.dma_start(t_[:], w2[m * P:(m + 1) * P, :])
                w2_sb.append(t_)
            b2_sb = cpool.tile([P, NDD], F32, tag="b2", name="b2_sb")
            nc.sync.dma_start(b2_sb[:], b2t[:])

            ht_tiles = [ht0]
            for g in range(1, NG):
                htg = ht_pool.tile([P, KC1, TG], F16, tag="ht", name=f"ht_g{g}")
                for k in range(KC1):
                    nc.sync.dma_start(
                        htg[:, k], hT[k * P:(k + 1) * P, g * TG:(g + 1) * TG])
                ht_tiles.append(htg)

            if explicit_ldw:
                # prime the PE weight registers; every matmul below runs
                # ldweights=False with the NEXT stationary loaded one
                # ahead (the weight port is double-buffered, so the load
                # overlaps the previous matmul's moving stream)
                nc.tensor.ldweights(w1_sb[0][:, 0:P])

            def _next_l1(g, m, hf, k, halves):
                if k < KC1 - 1:
                    return w1_sb[k + 1][:, m * P:(m + 1) * P]
                if hf < halves - 1:
                    return w1_sb[0][:, m * P:(m + 1) * P]
                if m < MC1 - 1:
                    return w1_sb[0][:, (m + 1) * P:(m + 2) * P]
                return w2_sb[0][:, 0:P]

            def _next_l2(g, dd, hc):
                if hc < MC1 - 1:
                    return w2_sb[hc + 1][:, dd * P:(dd + 1) * P]
                if dd < NDD - 1:
                    return w2_sb[0][:, (dd + 1) * P:(dd + 2) * P]
                if g < NG - 1:
                    return w1_sb[0][:, 0:P]
                return None

            for g in range(NG):
                htg = ht_tiles[g]
                hid = hid_pool.tile([P, MC1, TG], F16, tag="hid",
                                    name=f"hid_g{g}")

                # --- L1: hid[m, :] = gelu(W1[:,m-chunk].T @ ht + b1) ---
                for m in range(MC1):
                    # group 0's first two m-chunks run as 2-wide
                    # half-sweeps so they only depend on the first hT
                    # token-half (see startup staging above)
                    halves = 2 if (g == 0 and m < 2) else 1
                    width = NTS // halves
                    for hf in range(halves):
                        pss = [
                            l1_psum.tile([P, BT], F32, tag="l1",
                                         name=f"l1_{g}_{m}_{hf}_{t}")
                            for t in range(width)
                        ]
                        t0 = hf * width
                        for k in range(KC1):
                            stat = w1_sb[k][:, m * P:(m + 1) * P]
                            for t in range(width):
                                mi = nc.tensor.matmul(
                                    pss[t][:], stat,
                                    htg[:, k,
                                        (t0 + t) * BT:(t0 + t + 1) * BT],
                                    start=(k == 0), stop=(k == KC1 - 1),
                                )
                                if explicit_ldw:
                                    mi.ins.ldweights = False
                                    if t == 0:
                                        ns_ = _next_l1(g, m, hf, k, halves)
                                        nc.tensor.ldweights(ns_)
                                elif t > 0:
                                    mi.ins.ldweights = False
                        for t in range(width):
                            nc.scalar.activation(
                                hid[:, m, (t0 + t) * BT:(t0 + t + 1) * BT],
                                pss[t][:],
                                mybir.ActivationFunctionType.Gelu,
                                bias=b1_sb[:, m:m + 1], scale=1.0,
                            )

                # --- L2 (transposed): feT[dd] = W2[:, dd-chunk].T @ hid ---
                # RS granularity: one ReduceScatter per dd-PAIR ([2*P, TG]
                # bf16 = 1 MB input) — small enough to overlap, few enough
                # that per-op rendezvous cost stays well under compute.
                # The very last pair is split into two single-dd chunks so
                # the drain tail after the final matmuls is halved.
                chunks = _rs_chunks(ndd_rs)[g]
                dd2chunk = {}
                for ci, (dd0, ndds, row0) in enumerate(chunks):
                    for dd_ in range(dd0, dd0 + ndds):
                        dd2chunk[dd_] = (ci, dd0, ndds, row0)
                fe_dram = None
                for dd in range(NDD):
                    ci, dd0, ndds, row0 = dd2chunk[dd]
                    if dd == dd0 and use_rs:
                        fe_dram = dram_pool.tile(
                            [ndds, P, TG], fe_dt, tag="fe_dram",
                            name=f"fe_{g}_{ci}")
                    pss = [
                        l1_psum.tile([P, BT], F32, tag="l1",
                                     name=f"l2_{g}_{dd}_{t}")
                        for t in range(NTS)
                    ]
                    for hc in range(MC1):
                        stat = w2_sb[hc][:, dd * P:(dd + 1) * P]
                        for t in range(NTS):
                            mi = nc.tensor.matmul(
                                pss[t][:], stat,
                                hid[:, hc, t * BT:(t + 1) * BT],
                                start=(hc == 0), stop=(hc == MC1 - 1),
                            )
                            if explicit_ldw:
                                mi.ins.ldweights = False
                                if t == 0:
                                    ns_ = _next_l2(g, dd, hc)
                                    if ns_ is not None:
                                        nc.tensor.ldweights(ns_)
                            elif t > 0:
                                mi.ins.ldweights = False
                    for t in range(NTS):
                        fe_sb = fe_pool.tile(
                            [P, BT], fe_dt, tag="fe_sb",
                            name=f"fe_sb_{g}_{dd}_{t}")
                        nc.scalar.activation(
                            fe_sb[:], pss[t][:],
                            mybir.ActivationFunctionType.Identity,
                            bias=b2_sb[:, dd:dd + 1], scale=1.0,
                        )
                        if use_rs:
                            nc.sync.dma_start(
                                fe_dram[dd - dd0, :, t * BT:(t + 1) * BT],
                                fe_sb[:])
                        else:
                            r0 = (g * NDD + dd) * P
                            nc.sync.dma_start(
                                out[r0:r0 + P, t * BT:(t + 1) * BT], fe_sb[:])
                    if dd == dd0 + ndds - 1 and use_rs:
                        rpc = ndds * P // NCORES
                        rs_out = dram_pool.tile(
                            [rpc, TG], fe_dt, tag="rs_dram",
                            name=f"rs_{g}_{ci}")
                        nc.gpsimd.collective_compute(
                            "ReduceScatter",
                            mybir.AluOpType.add,
                            replica_groups=[list(range(NCORES))],
                            ins=[fe_dram[:]],
                            outs=[rs_out[:]],
                        )
                        nc.sync.dma_start(
                            out[row0:row0 + rpc, :], rs_out[:])

    nc.finalize()
    return nc


def _prepare_in_maps_v2(inputs):
    h = np.ascontiguousarray(np.asarray(inputs["h"], dtype=np.float32))
    hT16 = np.ascontiguousarray(h.T.astype(np.float16))  # [IN, B]
    gate_logits = np.asarray(inputs["gate_logits"], dtype=np.float64)
    W1 = np.asarray(inputs["W1"], dtype=np.float32)
    b1 = np.asarray(inputs["b1"], dtype=np.float32)
    W2 = np.asarray(inputs["W2"], dtype=np.float32)
    b2 = np.asarray(inputs["b2"], dtype=np.float32)

    z = np.exp(gate_logits - gate_logits.max())
    probs = (z / z.sum()).astype(np.float32)

    in_maps = []
    for e in range(NCORES):
        w1_e = np.ascontiguousarray(W1[e].astype(np.float16))       # [IN, H]
        b1t_e = np.ascontiguousarray(b1[e].reshape(MC1, P).T)       # [P, MC1]
        w2_e = np.ascontiguousarray(
            (W2[e] * probs[e]).astype(np.float16))                  # [H, D]
        b2t_e = np.ascontiguousarray(
            (b2[e] * probs[e]).reshape(NDD, P).T)                   # [P, NDD]
        in_maps.append(
            {"ht": hT16, "w1": w1_e, "b1t": b1t_e,
             "w2": w2_e, "b2t": b2t_e}
        )
    return in_maps


def _reassemble_v2(results, ndd_rs=2):
    # RS chunk (g, ci) covers dds [dd0, dd0+ndds) as a flat [ndds*P, TG]
    # buffer; core r receives rows r*rpc..(r+1)*rpc of it.
    final = np.empty((B, D), dtype=np.float32)
    sched = _rs_chunks(ndd_rs)
    for r in range(NCORES):
        o = np.asarray(results[r]["out"], dtype=np.float32)  # [256, TG]
        for g in range(NG):
            for dd0, ndds, row0 in sched[g]:
                rpc = ndds * P // NCORES
                blk = o[row0:row0 + rpc, :]                  # [rpc, TG]
                dd = dd0 + (r * rpc) // P
                d0 = dd * P + (r * rpc) % P
                final[g * TG:(g + 1) * TG, d0:d0 + rpc] = blk.T
    return final


# Best measured config: bf16 ReduceScatter (fp16 collectives hit a slow
# path; bf16 runs at full rate and halves the bytes), one RS per
# dd-pair (8 ops of 1 MB). Adds ~5e-3 rel err from bf16 partial-sum
# rounding -- well under the 2e-2 budget.
BEST_CFG = dict(fe_dt=mybir.dt.bfloat16, use_rs=True, ndd_rs="taper",
                shared_psum=True)


def _run_v2(inputs, trace=False):
    from concourse.bass_utils import run_bass_kernel_spmd

    in_maps = _prepare_in_maps_v2(inputs)
    nc = _get_nc("v2")
    res = run_bass_kernel_spmd(nc, in_maps, list(range(NCORES)), trace=trace)
    final = _reassemble_v2(res.results, ndd_rs=BEST_CFG["ndd_rs"])
    return final, res


def _get_nc(mm_dtype_name):
    key = mm_dtype_name
    if key not in _CACHE:
        if key == "v2":
            _CACHE[key] = build_v2(**BEST_CFG)
        else:
            _CACHE[key] = build(mm_dtype_name)
    return _CACHE[key]


def _prepare_in_maps(inputs, mm_dtype_name="float16"):
    import ml_dtypes

    np_mm = {"bfloat16": ml_dtypes.bfloat16, "float16": np.float16}.get(
        mm_dtype_name, np.float32
    )
    bf16 = np_mm != np.float32
    h = np.ascontiguousarray(np.asarray(inputs["h"], dtype=np.float32))
    if bf16:
        h = np.ascontiguousarray(h.T.astype(np_mm))  # pre-transposed [IN, B]
    gate_logits = np.asarray(inputs["gate_logits"], dtype=np.float64)
    W1 = np.asarray(inputs["W1"], dtype=np.float32)
    b1 = np.asarray(inputs["b1"], dtype=np.float32)
    W2 = np.asarray(inputs["W2"], dtype=np.float32)
    b2 = np.asarray(inputs["b2"], dtype=np.float32)

    # gate: softmax over E (uniform for zero logits); fold into W2/b2 per expert
    z = np.exp(gate_logits - gate_logits.max())
    probs = (z / z.sum()).astype(np.float32)

    in_maps = []
    for e in range(NCORES):
        w1_e = np.ascontiguousarray(W1[e].astype(np_mm))         # [IN, H]
        b1t_e = np.ascontiguousarray(b1[e].reshape(MC1, P).T)    # [P, MC1]
        w2_e = np.ascontiguousarray((W2[e] * probs[e]).astype(np_mm))  # [H, D]
        b2b_e = np.ascontiguousarray(
            np.broadcast_to(b2[e] * probs[e], (P, D))
        )
        in_maps.append(
            {("ht" if bf16 else "h"): h, "w1": w1_e, "b1t": b1t_e,
             "w2": w2_e, "b2b": b2b_e}
        )
    return in_maps


def _reassemble(results):
    # Reassemble. Chunks: tiles 0..NBT-2 are one 512-row RS each (64 rows per
    # core); the final tile is two 256-row RS (32 rows per core). Core r's
    # shard of a chunk starting at global row g0 with rows_per_core rpc lands
    # at final[g0 + r*rpc : g0 + (r+1)*rpc].
    chunks = []          # (global_row0, out_row0, rows_per_core)
    out_pos = 0
    for t in range(NBT):
        nhalves = 2 if t == NBT - 1 else 1
        rows = BT // nhalves
        for half in range(nhalves):
            rpc = rows // NCORES
            chunks.append((t * BT + half * rows, out_pos, rpc))
            out_pos += rpc
    final = np.empty((B, D), dtype=np.float32)
    for r in range(NCORES):
        o = results[r]["out"]
        for g0, o0, rpc in chunks:
            final[g0 + r * rpc: g0 + (r + 1) * rpc] = o[o0: o0 + rpc]
    return final


def _run(inputs, mm_dtype_name="float16", trace=False):
    from concourse.bass_utils import run_bass_kernel_spmd

    in_maps = _prepare_in_maps(inputs, mm_dtype_name)
    nc = _get_nc(mm_dtype_name)
    res = run_bass_kernel_spmd(nc, in_maps, list(range(NCORES)), trace=trace)
    final = _reassemble(res.results)
    return final, res


def kernel(**inputs):
    mm_dtype_name = os.environ.get("MOE_MM_DTYPE", "v2")
    if mm_dtype_name == "v2":
        final, _ = _run_v2(inputs, trace=False)
    else:
        final, _ = _run(inputs, mm_dtype_name=mm_dtype_name, trace=False)
    return final

